# revision 29
# baseline (speedup 1.0000x reference)
"""AttentionAggregation (segment softmax pooling) on 8 Trainium2 cores.

Math (per reference):
    s = tanh(x @ W1 + b1) @ W2 + b2            # [N]
    alpha = segment_softmax(s, batch_idx)      # [N, 1]
    out = segment_sum(x * alpha, batch_idx)    # [4096, 128]
Returns (out, alpha).

Strategy (v4):
  - batch_idx is sorted; 4096 segments -> 512 per core.  Per core the
    segments are permuted into two classes: 416 "A" slots padded to 512
    rows and 96 "B" slots padded to 640 rows (true lengths are 412..578),
    cutting padded bytes ~16% vs uniform 640.  Host uploads two bf16
    copies of x per core (transposed chunks for the score matmuls, natural
    chunks for the pooling matmuls); the device program is fully static
    and identical across cores.
  - exp(s) without max-subtraction (scores are in [-1.2, 1.2] here).
  - Batches of 32 segments: W1-stationary h matmuls, one tanh per segment,
    s^T rows accumulated into a [32, smax] PSUM tile via selection
    stationaries (W2 embedded in column k of a zero [128,32]).  Softmax in
    the transposed layout on 32 lanes.  Alpha is transposed back to natural
    chunk layout with cps cheap PE transposes per batch.
  - Pooling matmuls (x chunk stationary, alpha column moving, N=1) are
    interleaved into the next batch's streaming matmuls so the PE activity
    monitor keeps the array at full clock.
"""

import numpy as np
import ml_dtypes

N = 2_000_000
D = 128
NSEG = 4096
NCORES = 8
SPC = NSEG // NCORES          # segments per core = 512
SEGB = 32                     # segments per batch
SMAX_A, CPS_A = 512, 4
SMAX_B, CPS_B = 640, 5
NBA = 13                      # A batches  (416 segments)
NBB = 3                       # B batches  (96 slots)
NSEG_A = NBA * SEGB           # 416
NSEG_B = NBB * SEGB           # 96
ROWS_A = NSEG_A * SMAX_A      # 212992
ROWS_B = NSEG_B * SMAX_B      # 61440
RPAD = ROWS_A + ROWS_B        # 274432 padded rows per core
MW = NBA * SMAX_A + NBB * SMAX_B   # mask/alpha width = 8576

BF16 = ml_dtypes.bfloat16

_CACHE = {}


def _batch_cfgs():
    """(slot0, smax, cps, x_col_off, m_col_off) per batch."""
    cfgs = []
    for bi in range(NBA):
        cfgs.append((bi * SEGB, SMAX_A, CPS_A,
                     bi * SEGB * SMAX_A, bi * SMAX_A))
    for bj in range(NBB):
        cfgs.append((NSEG_A + bj * SEGB, SMAX_B, CPS_B,
                     ROWS_A + bj * SEGB * SMAX_B, NBA * SMAX_A + bj * SMAX_B))
    return cfgs


def _build_nc():
    import concourse.bacc as bacc
    import concourse.mybir as mybir
    from concourse.tile import TileContext

    f32 = mybir.dt.float32
    bf16 = mybir.dt.bfloat16

    nc = bacc.Bacc(None, target_bir_lowering=False)

    xt = nc.dram_tensor("xt", [128, RPAD], bf16, kind="ExternalInput")
    xn = nc.dram_tensor("xn", [128, RPAD], bf16, kind="ExternalInput")
    maskt = nc.dram_tensor("maskt", [SEGB, MW], f32, kind="ExternalInput")
    w1 = nc.dram_tensor("w1", [128, 128], bf16, kind="ExternalInput")
    w2s = nc.dram_tensor("w2s", [128, SEGB * SEGB], bf16, kind="ExternalInput")
    id32 = nc.dram_tensor("id32", [32, 32], bf16, kind="ExternalInput")
    b1d = nc.dram_tensor("b1d", [128, 1], f32, kind="ExternalInput")
    outp = nc.dram_tensor("outp", [128, SPC], f32, kind="ExternalOutput")
    alph = nc.dram_tensor("alph", [SEGB, MW], f32, kind="ExternalOutput")

    TANH = mybir.ActivationFunctionType.Tanh
    EXP = mybir.ActivationFunctionType.Exp
    ADD = mybir.AluOpType.add
    AXX = mybir.AxisListType.X

    with TileContext(nc) as tc:
        with (
            tc.tile_pool(name="const", bufs=1) as pc,
            tc.tile_pool(name="xt_p", bufs=2) as px_t,
            tc.tile_pool(name="xn_p", bufs=2) as px_n,
            tc.tile_pool(name="tan_p", bufs=4) as ptan,
            tc.tile_pool(name="soft_p", bufs=1) as pe,
            tc.tile_pool(name="xfer_p", bufs=2) as pxf,
            tc.tile_pool(name="ps_h", bufs=2, space="PSUM") as pp_h,
            tc.tile_pool(name="ps_s", bufs=1, space="PSUM") as pp_s,
            tc.tile_pool(name="ps_at", bufs=1, space="PSUM") as pp_at,
            tc.tile_pool(name="ps_o", bufs=1, space="PSUM") as pp_o,
        ):
            w1_sb = pc.tile([128, 128], bf16)
            nc.sync.dma_start(w1_sb[:], w1[:, :])
            w2s_sb = pc.tile([128, SEGB * SEGB], bf16)
            nc.sync.dma_start(w2s_sb[:], w2s[:, :])
            id32_sb = pc.tile([32, 32], bf16)
            nc.sync.dma_start(id32_sb[:], id32[:, :])
            b1_sb = pc.tile([128, 1], f32)
            nc.sync.dma_start(b1_sb[:], b1d[:, :])
            out_sb = pc.tile([128, SPC], f32)

            cfgs = _batch_cfgs()

            class Batch:
                pass

            def emit_loads(cfg):
                slot0, smax, cps, xcol, mcol = cfg
                b = Batch()
                b.slot0, b.smax, b.cps = slot0, smax, cps
                bw = SEGB * smax
                b.xt = px_t.tile([128, SEGB * SMAX_B], bf16, tag="xt")
                qw = bw // 4
                for q in range(4):
                    nc.sync.dma_start(b.xt[:, q * qw:(q + 1) * qw],
                                      xt[:, xcol + q * qw:xcol + (q + 1) * qw])
                b.xn = px_n.tile([128, SEGB * SMAX_B], bf16, tag="xn")
                for q in range(4):
                    nc.scalar.dma_start(b.xn[:, q * qw:(q + 1) * qw],
                                        xn[:, xcol + q * qw:xcol + (q + 1) * qw])
                b.mask = pxf.tile([SEGB, SMAX_B], f32, tag="mask", name="mask")
                nc.scalar.dma_start(b.mask[:, :smax],
                                    maskt[:, mcol:mcol + smax])
                b.mcol = mcol
                b.sT_a = pp_s.tile([SEGB, 512], f32, tag="sT_a", name="sTa")
                b.sT_b = pp_s.tile([SEGB, 128], f32, tag="sT_b", name="sTb")
                b.tans = [None] * SEGB
                return b

            def h_tanh(b, k):
                base = k * b.smax
                h = pp_h.tile([128, SMAX_B], f32, tag="h")
                nc.tensor.matmul(h[:, :512], lhsT=w1_sb[:],
                                 rhs=b.xt[:, base:base + 512],
                                 start=True, stop=True)
                if b.smax > 512:
                    nc.tensor.matmul(h[:, 512:640], lhsT=w1_sb[:],
                                     rhs=b.xt[:, base + 512:base + 640],
                                     start=True, stop=True)
                tanhT = ptan.tile([128, SMAX_B], bf16)
                nc.scalar.activation(tanhT[:, :b.smax], h[:, :b.smax], TANH,
                                     bias=b1_sb[:, :1])
                b.tans[k] = tanhT[:, :b.smax]

            def h_tanh_pairA(b, k):
                """Two 512-row segments share one h tile and one tanh op.
                [128,1024] f32 spans exactly 2 PSUM banks; the two matmul
                slices are bank-aligned."""
                base = k * 512
                h = pp_h.tile([128, 2 * 512], f32, tag="h")
                nc.tensor.matmul(h[:, :512], lhsT=w1_sb[:],
                                 rhs=b.xt[:, base:base + 512],
                                 start=True, stop=True)
                nc.tensor.matmul(h[:, 512:], lhsT=w1_sb[:],
                                 rhs=b.xt[:, base + 512:base + 1024],
                                 start=True, stop=True)
                tanhT = ptan.tile([128, 2 * 512], bf16, tag="tanp")
                nc.scalar.activation(tanhT[:], h[:], TANH, bias=b1_sb[:, :1])
                b.tans[k] = tanhT[:, :512]
                b.tans[k + 1] = tanhT[:, 512:]

            def sT(b, k):
                sel = w2s_sb[:, k * SEGB:(k + 1) * SEGB]
                nc.tensor.matmul(b.sT_a[:, :], lhsT=sel,
                                 rhs=b.tans[k][:, :512],
                                 start=(k == 0), stop=(k == SEGB - 1))
                if b.smax > 512:
                    nc.tensor.matmul(b.sT_b[:, :], lhsT=sel,
                                     rhs=b.tans[k][:, 512:640],
                                     start=(k == 0), stop=(k == SEGB - 1))

            def emit_softmax(b):
                """Softmax on 32 lanes; alpha DMA out + bf16 alpha."""
                smax = b.smax
                em = pe.tile([SEGB, SMAX_B], f32, tag="em")
                nc.vector.tensor_tensor(out=em[:, :512], in0=b.sT_a[:],
                                        in1=b.mask[:, :512], op=ADD)
                if smax > 512:
                    nc.vector.tensor_tensor(out=em[:, 512:640], in0=b.sT_b[:],
                                            in1=b.mask[:, 512:640], op=ADD)
                nc.scalar.activation(em[:, :smax], em[:, :smax], EXP)
                sums = pe.tile([SEGB, 1], f32, tag="sums")
                nc.vector.tensor_reduce(out=sums[:], in_=em[:, :smax],
                                        axis=AXX, op=ADD)
                nc.vector.tensor_scalar_add(sums[:], sums[:], 1e-16)
                recip = pe.tile([SEGB, 1], f32, tag="recip")
                nc.vector.reciprocal(recip[:], sums[:])
                alpha_b = pe.tile([SEGB, SMAX_B], f32, tag="alpha_b")
                nc.vector.tensor_scalar_mul(alpha_b[:, :smax], em[:, :smax],
                                            recip[:, :1])
                nc.scalar.dma_start(alph[:, b.mcol:b.mcol + smax],
                                    alpha_b[:, :smax])
                b.ab = pxf.tile([SEGB, SMAX_B], bf16, tag="ab")
                nc.vector.tensor_copy(out=b.ab[:, :smax], in_=alpha_b[:, :smax])

            def emit_alpha_nat(b):
                """Transpose alpha back to natural chunk layout.
                an column t*32 + k holds segment k's chunk-t alpha."""
                at_ps = pp_at.tile([128, CPS_B * SEGB], bf16)
                for t in range(b.cps):
                    nc.tensor.transpose(
                        out=at_ps[:, t * SEGB:(t + 1) * SEGB],
                        in_=b.ab[:, t * 128:(t + 1) * 128],
                        identity=id32_sb[:])
                b.an = pxf.tile([128, CPS_B * SEGB], bf16, tag="an")
                nc.vector.tensor_copy(out=b.an[:, :b.cps * SEGB],
                                      in_=at_ps[:, :b.cps * SEGB])
                b.po = pp_o.tile([128, SEGB], f32, tag="po")

            def pool_seg(b, k):
                for t in range(b.cps):
                    nc.tensor.matmul(
                        b.po[:, k:k + 1],
                        lhsT=b.xn[:, (k * b.cps + t) * 128:
                                  (k * b.cps + t + 1) * 128],
                        rhs=b.an[:, t * SEGB + k:t * SEGB + k + 1],
                        start=(k == 0 and t == 0),
                        stop=(k == SEGB - 1 and t == b.cps - 1))

            def pool_finish(b):
                nc.vector.tensor_copy(
                    out=out_sb[:, b.slot0:b.slot0 + SEGB], in_=b.po[:])

            LAG = 10
            prev = None
            for cfg in cfgs:
                b = emit_loads(cfg)
                isA = b.smax == SMAX_A
                if isA:
                    h_tanh_pairA(b, 0)
                    h_tanh_pairA(b, 2)
                else:
                    h_tanh(b, 0)
                    h_tanh(b, 1)
                for k in range(SEGB):
                    if isA:
                        if k % 2 == 0 and k + 4 < SEGB:
                            h_tanh_pairA(b, k + 4)
                    elif k + 2 < SEGB:
                        h_tanh(b, k + 2)
                    sT(b, k)
                    if prev is not None:
                        if k == 6:
                            # PE streams queued above hide the softmax
                            # DVE chain this depends on
                            emit_alpha_nat(prev)
                        elif k >= LAG:
                            pool_seg(prev, k - LAG)
                if prev is not None:
                    for k in range(SEGB - LAG, SEGB):
                        pool_seg(prev, k)
                    pool_finish(prev)
                emit_softmax(b)
                prev = b
            emit_alpha_nat(prev)
            for k in range(SEGB):
                pool_seg(prev, k)
            pool_finish(prev)

            nc.scalar.dma_start(outp[:, :], out_sb[:])

    nc.finalize()
    return nc


def _prep_inputs(x, W1, b1, W2, b2, batch_idx):
    """Host-side shard/permute/pad/swizzle."""
    x = np.ascontiguousarray(x, dtype=np.float32)
    bi = np.ascontiguousarray(batch_idx).astype(np.int64)
    starts = np.searchsorted(bi, np.arange(NSEG + 1)).astype(np.int64)
    lens = np.diff(starts)
    assert lens.max() <= SMAX_B, f"segment length {lens.max()} > {SMAX_B}"

    b2f = float(np.asarray(b2).reshape(-1)[0])
    w1b = np.ascontiguousarray(W1, dtype=np.float32).astype(BF16)
    w2sel = np.zeros((128, SEGB * SEGB), dtype=BF16)
    for k in range(SEGB):
        w2sel[:, k * SEGB + k] = np.asarray(W2, dtype=np.float32).reshape(128)
    id32v = np.eye(32, dtype=np.float32).astype(BF16)
    b1f = np.ascontiguousarray(b1, dtype=np.float32).reshape(128, 1)

    in_maps, perms = [], []
    for c in range(NCORES):
        s0 = c * SPC
        lens_c = lens[s0:s0 + SPC]
        assert (lens_c > SMAX_A).sum() <= NSEG_B, \
            f"core {c}: {(lens_c > SMAX_A).sum()} segments >512 rows > {NSEG_B}"
        longest = np.argsort(-lens_c, kind="stable")
        segB = np.sort(longest[:NSEG_B])     # 96 longest -> B slots (640 rows)
        segA = np.sort(longest[NSEG_B:])     # remaining 416 -> A slots (512)
        assert lens_c[segA].max() <= SMAX_A
        perm = np.concatenate([segA, segB]).astype(np.int64)
        perms.append(perm)

        xb = x[starts[s0]:starts[s0 + SPC]].astype(BF16)

        def build(slots, smax):
            nseg = len(slots)
            pad = np.zeros((nseg, smax, 128), dtype=BF16)
            ln = np.where(slots >= 0, lens_c[np.maximum(slots, 0)], 0)
            valid = np.arange(smax)[None, :] < ln[:, None]
            idx = [np.arange(starts[s0 + s] - starts[s0],
                             starts[s0 + s] - starts[s0] + lens_c[s])
                   for s in slots if s >= 0]
            if idx:
                src = np.concatenate(idx)
                pad[valid] = xb[src]
            mask = np.where(valid, b2f, np.float32(-1e30)).astype(np.float32)
            return pad, mask

        padA, maskA = build(perm[:NSEG_A], SMAX_A)
        padB, maskB = build(perm[NSEG_A:], SMAX_B)

        flat = np.concatenate([padA.reshape(-1, 128), padB.reshape(-1, 128)])
        xt_host = np.ascontiguousarray(flat.T)           # [128, RPAD]
        xn_host = np.ascontiguousarray(
            flat.reshape(-1, 128, 128).transpose(1, 0, 2)).reshape(128, RPAD)
        mask_host = np.concatenate(
            [maskA.reshape(NBA, SEGB, SMAX_A).transpose(1, 0, 2).reshape(SEGB, -1),
             maskB.reshape(NBB, SEGB, SMAX_B).transpose(1, 0, 2).reshape(SEGB, -1)],
            axis=1)
        in_maps.append({
            "xt": xt_host, "xn": xn_host,
            "maskt": np.ascontiguousarray(mask_host),
            "w1": w1b, "w2s": w2sel, "id32": id32v, "b1d": b1f,
        })
    return in_maps, starts, lens, perms


def _install_trace_shim():
    """Optional: register the axon NTFF profile hook so BASS_TRACE=1 works."""
    try:
        import sys
        import types
        if "antenv.axon_hooks" in sys.modules:
            return
        mod = types.ModuleType("antenv.axon_hooks")
        _h = [None]
        mod.set_axon_ntff_profile_hook = lambda h: _h.__setitem__(0, h)
        mod.get_axon_ntff_profile_hook = lambda: _h[0]
        sys.modules["antenv.axon_hooks"] = mod
        try:
            import antenv
            antenv.axon_hooks = mod
        except ImportError:
            pass
        sys.path.insert(0, "/root/.axon_site")
        from trn_agent_boot.trn_boot import _ntff_profile_via_ctypes
        hook = _ntff_profile_via_ctypes("/opt/axon/libaxon_pjrt.so")
        if hook is not None:
            mod.set_axon_ntff_profile_hook(hook)
        import concourse.bass_utils as bu
        bu.upload_artifacts = lambda tmpdir: tmpdir  # no bucket in this env
    except Exception:
        pass


def kernel(x, W1, b1, W2, b2, batch_idx, dim_size):
    import os
    if os.environ.get("BASS_TRACE"):
        _install_trace_shim()
    from concourse.bass_utils import run_bass_kernel_spmd

    if "nc" not in _CACHE:
        _CACHE["nc"] = _build_nc()
    nc = _CACHE["nc"]

    in_maps, starts, lens, perms = _prep_inputs(x, W1, b1, W2, b2, batch_idx)
    res = run_bass_kernel_spmd(nc, in_maps, core_ids=list(range(NCORES)))
    _CACHE["last_result"] = res

    cfgs = _batch_cfgs()
    out = np.zeros((NSEG, 128), dtype=np.float32)
    alpha = np.zeros((N,), dtype=np.float32)
    for c in range(NCORES):
        s0 = c * SPC
        r = res.results[c]
        perm = perms[c]
        ot = r["outp"]                      # [128, 512] slots
        al = r["alph"]                      # [32, MW]
        for slot0, smax, cps, xcol, mcol in cfgs:
            for k in range(SEGB):
                s = perm[slot0 + k]
                if s < 0:
                    continue
                out[s0 + s] = ot[:, slot0 + k]
                ln = lens[s0 + s]
                alpha[starts[s0 + s]:starts[s0 + s] + ln] = al[k, mcol:mcol + ln]
    return out, alpha[:, None]


# revision 31
# speedup vs baseline: 1.0247x; 1.0247x over previous
"""AttentionAggregation (segment softmax pooling) on 8 Trainium2 cores.

Math (per reference):
    s = tanh(x @ W1 + b1) @ W2 + b2            # [N]
    alpha = segment_softmax(s, batch_idx)      # [N, 1]
    out = segment_sum(x * alpha, batch_idx)    # [4096, 128]
Returns (out, alpha).

Strategy (v4):
  - batch_idx is sorted; 4096 segments -> 512 per core.  Per core the
    segments are permuted into two classes: 416 "A" slots padded to 512
    rows and 96 "B" slots padded to 640 rows (true lengths are 412..578),
    cutting padded bytes ~16% vs uniform 640.  Host uploads two bf16
    copies of x per core (transposed chunks for the score matmuls, natural
    chunks for the pooling matmuls); the device program is fully static
    and identical across cores.
  - exp(s) without max-subtraction (scores are in [-1.2, 1.2] here).
  - Batches of 32 segments: W1-stationary h matmuls, one tanh per segment,
    s^T rows accumulated into a [32, smax] PSUM tile via selection
    stationaries (W2 embedded in column k of a zero [128,32]).  Softmax in
    the transposed layout on 32 lanes.  Alpha is transposed back to natural
    chunk layout with cps cheap PE transposes per batch.
  - Pooling matmuls (x chunk stationary, alpha column moving, N=1) are
    interleaved into the next batch's streaming matmuls so the PE activity
    monitor keeps the array at full clock.
"""

import numpy as np
import ml_dtypes

N = 2_000_000
D = 128
NSEG = 4096
NCORES = 8
SPC = NSEG // NCORES          # segments per core = 512
SEGB = 32                     # segments per batch
SMAX_A, CPS_A = 512, 4
SMAX_B, CPS_B = 640, 5
NBA = 13                      # A batches  (416 segments)
NBB = 3                       # B batches  (96 slots)
NSEG_A = NBA * SEGB           # 416
NSEG_B = NBB * SEGB           # 96
ROWS_A = NSEG_A * SMAX_A      # 212992
ROWS_B = NSEG_B * SMAX_B      # 61440
RPAD = ROWS_A + ROWS_B        # 274432 padded rows per core
MW = NBA * SMAX_A + NBB * SMAX_B   # mask/alpha width = 8576

BF16 = ml_dtypes.bfloat16

_CACHE = {}


def _batch_cfgs():
    """(slot0, smax, cps, x_col_off, m_col_off) per batch."""
    cfgs = []
    for bi in range(NBA):
        cfgs.append((bi * SEGB, SMAX_A, CPS_A,
                     bi * SEGB * SMAX_A, bi * SMAX_A))
    for bj in range(NBB):
        cfgs.append((NSEG_A + bj * SEGB, SMAX_B, CPS_B,
                     ROWS_A + bj * SEGB * SMAX_B, NBA * SMAX_A + bj * SMAX_B))
    return cfgs


def _build_nc():
    import concourse.bacc as bacc
    import concourse.mybir as mybir
    from concourse.tile import TileContext

    f32 = mybir.dt.float32
    bf16 = mybir.dt.bfloat16

    nc = bacc.Bacc(None, target_bir_lowering=False)

    xt = nc.dram_tensor("xt", [128, RPAD], bf16, kind="ExternalInput")
    xn = nc.dram_tensor("xn", [128, RPAD], bf16, kind="ExternalInput")
    maskt = nc.dram_tensor("maskt", [SEGB, MW], f32, kind="ExternalInput")
    w1 = nc.dram_tensor("w1", [128, 128], bf16, kind="ExternalInput")
    w2s = nc.dram_tensor("w2s", [128, SEGB * SEGB], bf16, kind="ExternalInput")
    id32 = nc.dram_tensor("id32", [32, 32], bf16, kind="ExternalInput")
    b1d = nc.dram_tensor("b1d", [128, 1], f32, kind="ExternalInput")
    outp = nc.dram_tensor("outp", [128, SPC], f32, kind="ExternalOutput")
    alph = nc.dram_tensor("alph", [SEGB, MW], f32, kind="ExternalOutput")

    TANH = mybir.ActivationFunctionType.Tanh
    EXP = mybir.ActivationFunctionType.Exp
    ADD = mybir.AluOpType.add
    AXX = mybir.AxisListType.X

    with TileContext(nc) as tc:
        with (
            tc.tile_pool(name="const", bufs=1) as pc,
            tc.tile_pool(name="xt_p", bufs=2) as px_t,
            tc.tile_pool(name="xn_p", bufs=2) as px_n,
            tc.tile_pool(name="tan_p", bufs=4) as ptan,
            tc.tile_pool(name="soft_p", bufs=1) as pe,
            tc.tile_pool(name="xfer_p", bufs=2) as pxf,
            tc.tile_pool(name="ps_h", bufs=2, space="PSUM") as pp_h,
            tc.tile_pool(name="ps_s", bufs=1, space="PSUM") as pp_s,
            tc.tile_pool(name="ps_at", bufs=1, space="PSUM") as pp_at,
            tc.tile_pool(name="ps_o", bufs=1, space="PSUM") as pp_o,
        ):
            w1_sb = pc.tile([128, 128], bf16)
            nc.sync.dma_start(w1_sb[:], w1[:, :])
            w2s_sb = pc.tile([128, SEGB * SEGB], bf16)
            nc.sync.dma_start(w2s_sb[:], w2s[:, :])
            id32_sb = pc.tile([32, 32], bf16)
            nc.sync.dma_start(id32_sb[:], id32[:, :])
            b1_sb = pc.tile([128, 1], f32)
            nc.sync.dma_start(b1_sb[:], b1d[:, :])
            out_sb = pc.tile([128, SPC], f32)

            cfgs = _batch_cfgs()

            class Batch:
                pass

            def emit_loads(cfg):
                slot0, smax, cps, xcol, mcol = cfg
                b = Batch()
                b.slot0, b.smax, b.cps = slot0, smax, cps
                bw = SEGB * smax
                b.xt = px_t.tile([128, SEGB * SMAX_B], bf16, tag="xt")
                qw = bw // 4
                for q in range(4):
                    nc.sync.dma_start(b.xt[:, q * qw:(q + 1) * qw],
                                      xt[:, xcol + q * qw:xcol + (q + 1) * qw])
                b.xn = px_n.tile([128, SEGB * SMAX_B], bf16, tag="xn")
                for q in range(4):
                    nc.scalar.dma_start(b.xn[:, q * qw:(q + 1) * qw],
                                        xn[:, xcol + q * qw:xcol + (q + 1) * qw])
                b.mask = pxf.tile([SEGB, SMAX_B], f32, tag="mask", name="mask")
                nc.scalar.dma_start(b.mask[:, :smax],
                                    maskt[:, mcol:mcol + smax])
                b.mcol = mcol
                b.sT_a = pp_s.tile([SEGB, 512], f32, tag="sT_a", name="sTa")
                b.sT_b = pp_s.tile([SEGB, 128], f32, tag="sT_b", name="sTb")
                b.tans = [None] * SEGB
                return b

            def h_tanh(b, k):
                base = k * b.smax
                h = pp_h.tile([128, SMAX_B], f32, tag="h")
                nc.tensor.matmul(h[:, :512], lhsT=w1_sb[:],
                                 rhs=b.xt[:, base:base + 512],
                                 start=True, stop=True)
                if b.smax > 512:
                    nc.tensor.matmul(h[:, 512:640], lhsT=w1_sb[:],
                                     rhs=b.xt[:, base + 512:base + 640],
                                     start=True, stop=True)
                tanhT = ptan.tile([128, SMAX_B], bf16)
                nc.scalar.activation(tanhT[:, :b.smax], h[:, :b.smax], TANH,
                                     bias=b1_sb[:, :1])
                b.tans[k] = tanhT[:, :b.smax]

            def h_tanh_pairA(b, k):
                """Two 512-row segments share one h tile and one tanh op.
                [128,1024] f32 spans exactly 2 PSUM banks; the two matmul
                slices are bank-aligned."""
                base = k * 512
                h = pp_h.tile([128, 2 * 512], f32, tag="h")
                nc.tensor.matmul(h[:, :512], lhsT=w1_sb[:],
                                 rhs=b.xt[:, base:base + 512],
                                 start=True, stop=True)
                nc.tensor.matmul(h[:, 512:], lhsT=w1_sb[:],
                                 rhs=b.xt[:, base + 512:base + 1024],
                                 start=True, stop=True)
                tanhT = ptan.tile([128, 2 * 512], bf16, tag="tanp")
                nc.scalar.activation(tanhT[:], h[:], TANH, bias=b1_sb[:, :1])
                b.tans[k] = tanhT[:, :512]
                b.tans[k + 1] = tanhT[:, 512:]

            def sT(b, k):
                sel = w2s_sb[:, k * SEGB:(k + 1) * SEGB]
                nc.tensor.matmul(b.sT_a[:, :], lhsT=sel,
                                 rhs=b.tans[k][:, :512],
                                 start=(k == 0), stop=(k == SEGB - 1))
                if b.smax > 512:
                    nc.tensor.matmul(b.sT_b[:, :], lhsT=sel,
                                     rhs=b.tans[k][:, 512:640],
                                     start=(k == 0), stop=(k == SEGB - 1))

            def emit_softmax(b):
                """Softmax on 32 lanes; alpha DMA out + bf16 alpha."""
                smax = b.smax
                em = pe.tile([SEGB, SMAX_B], f32, tag="em")
                nc.vector.tensor_tensor(out=em[:, :512], in0=b.sT_a[:],
                                        in1=b.mask[:, :512], op=ADD)
                if smax > 512:
                    nc.vector.tensor_tensor(out=em[:, 512:640], in0=b.sT_b[:],
                                            in1=b.mask[:, 512:640], op=ADD)
                nc.scalar.activation(em[:, :smax], em[:, :smax], EXP)
                sums = pe.tile([SEGB, 1], f32, tag="sums")
                nc.vector.tensor_reduce(out=sums[:], in_=em[:, :smax],
                                        axis=AXX, op=ADD)
                nc.vector.tensor_scalar_add(sums[:], sums[:], 1e-16)
                recip = pe.tile([SEGB, 1], f32, tag="recip")
                nc.vector.reciprocal(recip[:], sums[:])
                alpha_b = pe.tile([SEGB, SMAX_B], f32, tag="alpha_b")
                nc.vector.tensor_scalar_mul(alpha_b[:, :smax], em[:, :smax],
                                            recip[:, :1])
                nc.scalar.dma_start(alph[:, b.mcol:b.mcol + smax],
                                    alpha_b[:, :smax])
                b.ab = pxf.tile([SEGB, SMAX_B], bf16, tag="ab")
                nc.vector.tensor_copy(out=b.ab[:, :smax], in_=alpha_b[:, :smax])

            def emit_alpha_nat(b):
                """Transpose alpha back to natural chunk layout.
                an column t*32 + k holds segment k's chunk-t alpha."""
                at_ps = pp_at.tile([128, CPS_B * SEGB], bf16)
                for t in range(b.cps):
                    nc.tensor.transpose(
                        out=at_ps[:, t * SEGB:(t + 1) * SEGB],
                        in_=b.ab[:, t * 128:(t + 1) * 128],
                        identity=id32_sb[:])
                b.an = pxf.tile([128, CPS_B * SEGB], bf16, tag="an")
                nc.vector.tensor_copy(out=b.an[:, :b.cps * SEGB],
                                      in_=at_ps[:, :b.cps * SEGB])
                b.po = pp_o.tile([128, SEGB], f32, tag="po")

            def pool_seg(b, k):
                for t in range(b.cps):
                    nc.tensor.matmul(
                        b.po[:, k:k + 1],
                        lhsT=b.xn[:, (k * b.cps + t) * 128:
                                  (k * b.cps + t + 1) * 128],
                        rhs=b.an[:, t * SEGB + k:t * SEGB + k + 1],
                        start=(k == 0 and t == 0),
                        stop=(k == SEGB - 1 and t == b.cps - 1))

            def pool_finish(b):
                nc.vector.tensor_copy(
                    out=out_sb[:, b.slot0:b.slot0 + SEGB], in_=b.po[:])

            LAG = 6
            prev = None
            for cfg in cfgs:
                b = emit_loads(cfg)
                isA = b.smax == SMAX_A
                if isA:
                    h_tanh_pairA(b, 0)
                    h_tanh_pairA(b, 2)
                else:
                    h_tanh(b, 0)
                    h_tanh(b, 1)
                for k in range(SEGB):
                    if isA:
                        if k % 2 == 0 and k + 4 < SEGB:
                            h_tanh_pairA(b, k + 4)
                    elif k + 2 < SEGB:
                        h_tanh(b, k + 2)
                    sT(b, k)
                    if prev is not None:
                        if k == 4:
                            # PE streams queued above hide the softmax
                            # DVE chain this depends on
                            emit_alpha_nat(prev)
                        elif k >= LAG:
                            pool_seg(prev, k - LAG)
                if prev is not None:
                    for k in range(SEGB - LAG, SEGB):
                        pool_seg(prev, k)
                    pool_finish(prev)
                emit_softmax(b)
                prev = b
            emit_alpha_nat(prev)
            for k in range(SEGB):
                pool_seg(prev, k)
            pool_finish(prev)

            nc.scalar.dma_start(outp[:, :], out_sb[:])

    nc.finalize()
    return nc


def _prep_inputs(x, W1, b1, W2, b2, batch_idx):
    """Host-side shard/permute/pad/swizzle."""
    x = np.ascontiguousarray(x, dtype=np.float32)
    bi = np.ascontiguousarray(batch_idx).astype(np.int64)
    starts = np.searchsorted(bi, np.arange(NSEG + 1)).astype(np.int64)
    lens = np.diff(starts)
    assert lens.max() <= SMAX_B, f"segment length {lens.max()} > {SMAX_B}"

    b2f = float(np.asarray(b2).reshape(-1)[0])
    w1b = np.ascontiguousarray(W1, dtype=np.float32).astype(BF16)
    w2sel = np.zeros((128, SEGB * SEGB), dtype=BF16)
    for k in range(SEGB):
        w2sel[:, k * SEGB + k] = np.asarray(W2, dtype=np.float32).reshape(128)
    id32v = np.eye(32, dtype=np.float32).astype(BF16)
    b1f = np.ascontiguousarray(b1, dtype=np.float32).reshape(128, 1)

    in_maps, perms = [], []
    for c in range(NCORES):
        s0 = c * SPC
        lens_c = lens[s0:s0 + SPC]
        assert (lens_c > SMAX_A).sum() <= NSEG_B, \
            f"core {c}: {(lens_c > SMAX_A).sum()} segments >512 rows > {NSEG_B}"
        longest = np.argsort(-lens_c, kind="stable")
        segB = np.sort(longest[:NSEG_B])     # 96 longest -> B slots (640 rows)
        segA = np.sort(longest[NSEG_B:])     # remaining 416 -> A slots (512)
        assert lens_c[segA].max() <= SMAX_A
        perm = np.concatenate([segA, segB]).astype(np.int64)
        perms.append(perm)

        xb = x[starts[s0]:starts[s0 + SPC]].astype(BF16)

        def build(slots, smax):
            nseg = len(slots)
            pad = np.zeros((nseg, smax, 128), dtype=BF16)
            ln = np.where(slots >= 0, lens_c[np.maximum(slots, 0)], 0)
            valid = np.arange(smax)[None, :] < ln[:, None]
            idx = [np.arange(starts[s0 + s] - starts[s0],
                             starts[s0 + s] - starts[s0] + lens_c[s])
                   for s in slots if s >= 0]
            if idx:
                src = np.concatenate(idx)
                pad[valid] = xb[src]
            mask = np.where(valid, b2f, np.float32(-1e30)).astype(np.float32)
            return pad, mask

        padA, maskA = build(perm[:NSEG_A], SMAX_A)
        padB, maskB = build(perm[NSEG_A:], SMAX_B)

        flat = np.concatenate([padA.reshape(-1, 128), padB.reshape(-1, 128)])
        xt_host = np.ascontiguousarray(flat.T)           # [128, RPAD]
        xn_host = np.ascontiguousarray(
            flat.reshape(-1, 128, 128).transpose(1, 0, 2)).reshape(128, RPAD)
        mask_host = np.concatenate(
            [maskA.reshape(NBA, SEGB, SMAX_A).transpose(1, 0, 2).reshape(SEGB, -1),
             maskB.reshape(NBB, SEGB, SMAX_B).transpose(1, 0, 2).reshape(SEGB, -1)],
            axis=1)
        in_maps.append({
            "xt": xt_host, "xn": xn_host,
            "maskt": np.ascontiguousarray(mask_host),
            "w1": w1b, "w2s": w2sel, "id32": id32v, "b1d": b1f,
        })
    return in_maps, starts, lens, perms


def _install_trace_shim():
    """Optional: register the axon NTFF profile hook so BASS_TRACE=1 works."""
    try:
        import sys
        import types
        if "antenv.axon_hooks" in sys.modules:
            return
        mod = types.ModuleType("antenv.axon_hooks")
        _h = [None]
        mod.set_axon_ntff_profile_hook = lambda h: _h.__setitem__(0, h)
        mod.get_axon_ntff_profile_hook = lambda: _h[0]
        sys.modules["antenv.axon_hooks"] = mod
        try:
            import antenv
            antenv.axon_hooks = mod
        except ImportError:
            pass
        sys.path.insert(0, "/root/.axon_site")
        from trn_agent_boot.trn_boot import _ntff_profile_via_ctypes
        hook = _ntff_profile_via_ctypes("/opt/axon/libaxon_pjrt.so")
        if hook is not None:
            mod.set_axon_ntff_profile_hook(hook)
        import concourse.bass_utils as bu
        bu.upload_artifacts = lambda tmpdir: tmpdir  # no bucket in this env
    except Exception:
        pass


def kernel(x, W1, b1, W2, b2, batch_idx, dim_size):
    import os
    if os.environ.get("BASS_TRACE"):
        _install_trace_shim()
    from concourse.bass_utils import run_bass_kernel_spmd

    if "nc" not in _CACHE:
        _CACHE["nc"] = _build_nc()
    nc = _CACHE["nc"]

    in_maps, starts, lens, perms = _prep_inputs(x, W1, b1, W2, b2, batch_idx)
    res = run_bass_kernel_spmd(nc, in_maps, core_ids=list(range(NCORES)))
    _CACHE["last_result"] = res

    cfgs = _batch_cfgs()
    out = np.zeros((NSEG, 128), dtype=np.float32)
    alpha = np.zeros((N,), dtype=np.float32)
    for c in range(NCORES):
        s0 = c * SPC
        r = res.results[c]
        perm = perms[c]
        ot = r["outp"]                      # [128, 512] slots
        al = r["alph"]                      # [32, MW]
        for slot0, smax, cps, xcol, mcol in cfgs:
            for k in range(SEGB):
                s = perm[slot0 + k]
                if s < 0:
                    continue
                out[s0 + s] = ot[:, slot0 + k]
                ln = lens[s0 + s]
                alpha[starts[s0 + s]:starts[s0 + s] + ln] = al[k, mcol:mcol + ln]
    return out, alpha[:, None]
